# revision 23
# baseline (speedup 1.0000x reference)
"""Multi-head attention (RoPE) Trainium2 kernel.

Problem: B=2, T=2048, D_MODEL=1024, 16 heads x d_k=64, fp32 in/out.

Sharding: tensor-parallel over heads. Core c owns heads 2c, 2c+1:
  - wq/wk/wv rows [128c, 128c+128)  (column-split of the projections)
  - wo columns [128c, 128c+128)     (row-split of the output projection)
Each core emits a NORMALIZED fp16 partial of the output projection (softmax
denominators are applied on-chip); the host just sums the 8 partials.

On-chip dataflow per core (fp16 matmul operands, fp32 PSUM):
  xT [D=1024, tok=4096] (token-major b*2048+s) @ wT slices -> QT/KT/VT [128, 4096]
  RoPE on QT/KT in [d', tok] layout (tables precomputed host-side; the +-32
  partition swap is a SBUF-SBUF DMA).
  V transposed per 128-token tile on the PE to [tok, 64]-per-head tiles with
  a ones column appended (the 65th stationary column accumulates the softmax
  denominator into PSUM row 64 of the AV accumulator for free).
  Attention runs in 512-query chunks (8 chunks of (b, qi)).  Per key tile kt:
  both heads' score matmuls are row-tiled ((0,0)/(64,0)) into ONE [128,1024]
  PSUM tile (head h at columns h*512:(h+1)*512), so a single FD=1024 exp on
  ScalarE covers both heads.  Score tiles rotate through 3 PSUM slots so the
  PE can run ~2 key-tiles ahead of the exp stream.
  After the AV accumulation, row 64 of each accumulator holds the softmax
  denominator; a DVE reciprocal + a tiny K=2 PE matmul broadcasts 1/den to
  all 128 partitions, one tensor_mul normalizes the concatenated [128,512]
  attention output, and a single K=128 output projection (both heads fused)
  produces this core's fp16 partial, evicted with one fused DMA per chunk.

ScalarE (exp: 128 tiles x ~1.15us = ~147us) is the critical engine; all PE
work that is not on the scores->exp->AV chain (projections, RoPE, V
transposes) is streamed through a fill queue into the gaps of the attention
kt loop, and the previous chunk's output projection runs as a burst at each
chunk start, so the PE never idles long enough for the HAM clock gate to
re-throttle it to 1.2 GHz.
"""

import sys

sys.path.insert(0, "/opt/trn_rl_repo")

from collections import deque

import numpy as np

import concourse.bacc as bacc
import concourse.bass as bass
import concourse.tile as tile
from concourse import mybir
from concourse.masks import make_identity

F16 = mybir.dt.float16
F32 = mybir.dt.float32

B = 2
T = 2048
D = 1024
NTOK = B * T  # 4096
DK = 64
N_CORES = 8
QCH = 512  # query chunk
NCHUNK = NTOK // QCH  # 8
KT_N = T // 128  # 16 key tiles per batch


def _build_body(tc, xT, wqT, wkT, wvT, woT, ropeA, ropeB, outT):
    nc = tc.nc
    Exp = mybir.ActivationFunctionType.Exp

    const = tc.alloc_tile_pool(name="const", bufs=1)
    psum = tc.alloc_tile_pool(name="psum", bufs=1, space="PSUM")
    at = tc.alloc_tile_pool(name="attn", bufs=1)
    pp = tc.alloc_tile_pool(name="phasep", bufs=1)

    # ---------------- persistent tiles ----------------
    # Warm the ScalarE Exp table set (~2.7us) during the preamble DMAs.
    warm = const.tile([1, 8], F32)
    nc.vector.memset(warm, 0.0)
    warm_o = const.tile([1, 8], F16)
    nc.scalar.activation(warm_o, warm, Exp, scale=0.125)

    # Input loads are split across the two HWDGE rings (SP + ACT): the
    # ~0.6us per-DMA issue cost on a single queue otherwise serializes the
    # whole preamble.  ScalarE is idle until the first scores exist, so the
    # ACT ring is free real estate at startup.
    w_sb = {}
    for nm, w, eng in (
        ("wk", wkT, nc.scalar),
        ("wq", wqT, nc.scalar),
        ("wv", wvT, nc.sync),
    ):
        wt = const.tile([128, 8, 128], F16, name=f"{nm}sb")
        eng.dma_start(out=wt, in_=w.rearrange("(a p) m -> p a m", p=128))
        w_sb[nm] = wt
    ident = const.tile([128, 128], F16)
    make_identity(nc, ident)
    # den-broadcast selector: out[m,q] = sel[0,m]*rcp0[q] + sel[1,m]*rcp1[q]
    # (row 1 lives at partition 1, which DVE can't address -> build via DMA)
    sel = const.tile([2, 128], F32)
    ones128 = const.tile([1, 128], F32)
    nc.vector.memset(ones128, 1.0)
    nc.vector.memset(sel, 0.0)
    nc.sync.dma_start(out=sel[0:1, 0:64], in_=ones128[0:1, 0:64])
    nc.sync.dma_start(out=sel[1:2, 64:128], in_=ones128[0:1, 64:128])

    q_rot = const.tile([128, 4096], F16)
    k_rot = const.tile([128, 4096], F16)
    # per 128-token tile, per head: [V(0:64) | ones(64) | pad] fp16, all in
    # one big tile (slot 2*g+h) so a single memset plants the ones columns.
    vbig = const.tile([128, 64, 72], F16, name="vbig")
    nc.vector.memset(vbig, 1.0)

    def v_sb(g, h):
        return vbig[:, 2 * g + h, :]

    # x in SBUF: 8 k-tiles of [128, 4096], loaded in phase-sized quarters
    # (phase-0 projections start after the first 8 x 256 KB arrive).  The
    # RoPE tables repeat with period 64 partitions, so only half is read
    # from HBM; a SBUF-SBUF DMA duplicates it.  wo is not needed until the
    # first output projection (chunk 2) and loads last.
    xs = [pp.tile([128, 4096], F16, name=f"xs{k}") for k in range(8)]
    rA = const.tile([128, 4096], F16)
    rB = const.tile([128, 4096], F16)
    nc.scalar.dma_start(out=rA, in_=ropeA)
    nc.scalar.dma_start(out=rB, in_=ropeB)
    for half in range(2):
        cs = slice(half * 2048, (half + 1) * 2048)
        for k in range(8):
            eng = nc.sync if k % 2 == 0 else nc.scalar
            eng.dma_start(out=xs[k][:, cs], in_=xT[k * 128 : (k + 1) * 128, cs])
    wo_sb = const.tile([128, 1024], F16)
    nc.sync.dma_start(out=wo_sb, in_=woT)

    vt_raw = [pp.tile([128, 1024], F16, name=f"vtr{i}") for i in range(4)]

    # ---------------- projection / RoPE / V-transpose units ----------------
    def proj_quarter(wt, dst, t4, j, dst_off=None):
        # 4 matmuls: k in {2j, 2j+1} x h2; the accumulation group spans the
        # four quarters (start at k==0, stop at k==7); evict on the last.
        cs0 = t4 * 1024
        do = cs0 if dst_off is None else dst_off
        if j == 0:
            proj_quarter.cur = psum.tile(
                [128, 1024], F32, tag="proj", bufs=1, name="ps_pr"
            )
        ps = proj_quarter.cur
        for k in (2 * j, 2 * j + 1):
            for h2 in range(2):
                nc.tensor.matmul(
                    ps[:, h2 * 512 : (h2 + 1) * 512],
                    lhsT=wt[:, k, :],
                    rhs=xs[k][:, cs0 + h2 * 512 : cs0 + (h2 + 1) * 512],
                    start=(k == 0),
                    stop=(k == 7),
                    skip_group_check=True,
                )
        if j == 3:
            nc.vector.tensor_copy(dst[:, do : do + 1024], ps)

    def rope_chunk(raw, t4):
        # out = raw*A + swap(raw)*B, swap = +-32 partitions within a head
        cs = slice(t4 * 1024, (t4 + 1) * 1024)
        sw = pp.tile([128, 1024], F16, tag="sw", bufs=2, name="ropesw")
        for dst_p, src_p in ((0, 32), (32, 0), (64, 96), (96, 64)):
            nc.sync.dma_start(
                out=sw[dst_p : dst_p + 32, :], in_=raw[src_p : src_p + 32, cs]
            )
        t1 = pp.tile([128, 1024], F16, tag="t1", bufs=2, name="ropet1")
        nc.vector.tensor_mul(t1, raw[:, cs], rA[:, cs])
        nc.vector.tensor_mul(sw, sw, rB[:, cs])
        nc.vector.tensor_add(raw[:, cs], t1, sw)

    def v_transpose(t4, i):
        # vt_raw[t4] [d', tok] -> v_sb [tok128, d64] for token tile 8*t4+i
        pst = psum.tile([128, 1024], F32, tag="mm", bufs=2, name="ps_tr")
        tr = pst[:, 0:64].bitcast(F16)  # [128, 128] f16 view
        nc.tensor.transpose(tr, vt_raw[t4][:, i * 128 : (i + 1) * 128], ident)
        g = 8 * t4 + i
        nc.vector.tensor_copy(v_sb(g, 0)[:, 0:64], tr[:, 0:64])
        nc.vector.tensor_copy(v_sb(g, 1)[:, 0:64], tr[:, 64:128])

    # ---------------- fill + oproj queues ----------------
    # Every fill unit carries a (t4, part) key; `require` force-drains the
    # queue until the named parts are fully EMITTED, so a chunk's score/AV
    # instructions are never emitted before the projections they read --
    # emission order is what Tile derives dependencies from.
    fillq = deque()
    oq = deque()
    remaining = {}

    def fill(n):
        for _ in range(n):
            if not fillq:
                return
            key, fn = fillq.popleft()
            fn()
            if key is not None:
                remaining[key] -= 1

    def require(*keys):
        while any(remaining.get(k, 0) > 0 for k in keys):
            fill(1)

    def put(key, fn):
        remaining[key] = remaining.get(key, 0) + 1
        fillq.append((key, fn))

    def enqueue_phase(t4, parts):
        for p in parts:
            key = (t4, p)
            if p == "k":
                for j in range(4):
                    put(key, lambda j=j: proj_quarter(w_sb["wk"], k_rot, t4, j))
                put(key, lambda: rope_chunk(k_rot, t4))
            elif p == "q":
                for j in range(4):
                    put(key, lambda j=j: proj_quarter(w_sb["wq"], q_rot, t4, j))
                put(key, lambda: rope_chunk(q_rot, t4))
            elif p == "v":
                for j in range(4):
                    put(
                        key,
                        lambda j=j: proj_quarter(w_sb["wv"], vt_raw[t4], t4, j, dst_off=0),
                    )
                for i in range(0, 8, 2):
                    put(key, lambda i=i: (v_transpose(t4, i), v_transpose(t4, i + 1)))

    # ---------------- attention ----------------
    def chunk(c):
        b, qi = c // 4, c % 4
        qoff = b * T + qi * QCH

        exp_tiles = {}

        def s_exp(kt):
            koff = b * T + kt * 128
            pss = psum.tile([128, 1024], F32, tag="mm", bufs=2, name="ps_s")
            for hi in range(2):
                hs = slice(64 * hi, 64 * hi + 64)
                nc.tensor.matmul(
                    pss[:, hi * 512 : (hi + 1) * 512],
                    lhsT=k_rot[hs, koff : koff + 128],
                    rhs=q_rot[hs, qoff : qoff + QCH],
                    start=True,
                    stop=True,
                )
            e = at.tile([128, 1024], F16, tag="exp", bufs=4, name="exps")
            nc.scalar.activation(e, pss, Exp, scale=0.125)
            exp_tiles[kt] = e

        ps_o = []

        def av(kt):
            g = b * KT_N + kt
            e = exp_tiles.pop(kt)
            for hi in range(2):
                nc.tensor.matmul(
                    ps_o[hi],
                    lhsT=v_sb(g, hi)[:, 0:65],
                    rhs=e[:, hi * 512 : (hi + 1) * 512],
                    start=(kt == 0),
                    stop=(kt == KT_N - 1),
                    skip_group_check=True,
                )

        require((2 * b, "k"), (c // 2, "q"))
        s_exp(0)
        s_exp(1)
        # prior chunks' output-projection units run as fill work two chunks
        # later, when the den/normalize chain is guaranteed long-finished, so
        # no PE instruction in this chunk's queue ever head-of-line blocks.
        ndrain = len(oq) if c == NCHUNK - 1 else (1 if len(oq) >= 2 else 0)
        for _ in range(ndrain):
            for u in reversed(oq.popleft()):
                fillq.appendleft((None, u))
        ps_o.extend(
            psum.tile([65, QCH], F32, tag="o", bufs=2, name=f"ps_o{hi}")
            for hi in range(2)
        )
        # AV trails scores by TWO key-tiles so its exp input is always long
        # ready -- the PE never waits on ScalarE, keeping matmuls
        # back-to-back (streamed ~N/2.4 instead of isolated (398+N)/2.4).
        require((2 * b, "v"))
        s_exp(2)
        av(0)
        fill(1)
        for kt in range(3, KT_N):
            if kt == 8:
                require((2 * b + 1, "k"))
            if kt == 10:
                require((2 * b + 1, "v"))
            s_exp(kt)
            av(kt - 2)
            fill(2 if c >= NCHUNK - 2 else 1)
        av(KT_N - 2)
        av(KT_N - 1)

        # softmax denominators -> 1/den broadcast -> normalize -> oproj
        # denominator rows -> SBUF, spread across 128 partitions, one wide
        # reciprocal (8 elem/lane instead of 512 on one lane), then back into
        # the [2, 512] layout the broadcast matmul wants.
        dT = at.tile([128, 8], F32, tag="dT", bufs=3, name="dT")
        for hi in range(2):
            dtmp = at.tile([1, QCH], F32, tag=f"dtmp{hi}", bufs=3, name="dtmp")
            nc.vector.tensor_copy(dtmp, ps_o[hi][64:65, :])
            nc.sync.dma_start(out=dT[:, 4 * hi : 4 * hi + 4], in_=dtmp)
        rT = at.tile([128, 8], F32, tag="rT", bufs=3, name="rT")
        nc.vector.reciprocal(rT, dT)
        rcp2 = at.tile([2, QCH], F32, tag="rcp", bufs=3, name="rcp2")
        for hi in range(2):
            nc.sync.dma_start(out=rcp2[hi : hi + 1, :], in_=rT[:, 4 * hi : 4 * hi + 4])
        ocat = at.tile([128, QCH], F16, tag="ocat", bufs=3, name="ocat")
        nc.vector.tensor_copy(ocat[0:64, :], ps_o[0][0:64, :])
        oBt = at.tile([64, QCH], F16, tag="oBt", bufs=3, name="oBt")
        nc.vector.tensor_copy(oBt, ps_o[1][0:64, :])
        nc.sync.dma_start(out=ocat[64:128, :], in_=oBt)

        st = {}

        def o_pre():
            ps_b = psum.tile([128, QCH], F32, tag="mm", bufs=2, name="ps_b")
            nc.tensor.matmul(ps_b, lhsT=sel, rhs=rcp2, start=True, stop=True)
            ocn = at.tile([128, QCH], F16, tag="ocn", bufs=3, name="ocn")
            nc.vector.tensor_mul(ocn, ocat, ps_b)
            st["ocn"] = ocn
            st["ot"] = at.tile([128, 8, QCH], F16, tag="ot", bufs=3, name="ot")

        def o_nt(nt):
            ps_u = psum.tile([128, QCH], F32, tag="mm", bufs=2, name="ps_u")
            nc.tensor.matmul(
                ps_u,
                lhsT=wo_sb[:, nt * 128 : (nt + 1) * 128],
                rhs=st["ocn"],
                start=True,
                stop=True,
            )
            nc.vector.tensor_copy(st["ot"][:, nt, :], ps_u)
            if nt == 7:
                nc.sync.dma_start(
                    out=outT[:, qoff : qoff + QCH].rearrange(
                        "(nt p) q -> p nt q", nt=8
                    ),
                    in_=st["ot"],
                )

        oq.append([o_pre] + [(lambda nt=nt: o_nt(nt)) for nt in range(8)])

    # ---------------- schedule ----------------
    # Preamble: phase 0 fully + phase 1 K; phase 1 V leads the fill queue so
    # batch 0's second-half V tiles are ready by chunk 0's kt 8.
    for j in range(4):
        proj_quarter(w_sb["wk"], k_rot, 0, j)
    rope_chunk(k_rot, 0)
    for j in range(4):
        proj_quarter(w_sb["wq"], q_rot, 0, j)
    rope_chunk(q_rot, 0)
    for j in range(4):
        proj_quarter(w_sb["wv"], vt_raw[0], 0, j, dst_off=0)
    for i in range(0, 8, 2):
        v_transpose(0, i)
        v_transpose(0, i + 1)
    enqueue_phase(1, "kv")
    enqueue_phase(1, "q")
    enqueue_phase(2, "kv")
    enqueue_phase(2, "q")
    enqueue_phase(3, "kv")
    enqueue_phase(3, "q")

    for c in range(NCHUNK):
        chunk(c)
        if c == NCHUNK - 1:
            fill(10**9)
            while oq:
                for u in oq.popleft():
                    u()

    pp.release()
    at.release()
    const.release()
    psum.release()


_NC_CACHE = {}


def _build_program():
    if 0 in _NC_CACHE:
        return _NC_CACHE[0]
    nc = bacc.Bacc("TRN2", num_devices=N_CORES, debug=False)
    xT = nc.dram_tensor("xT", [D, NTOK], F16, kind="ExternalInput").ap()
    wqT = nc.dram_tensor("wqT", [D, 128], F16, kind="ExternalInput").ap()
    wkT = nc.dram_tensor("wkT", [D, 128], F16, kind="ExternalInput").ap()
    wvT = nc.dram_tensor("wvT", [D, 128], F16, kind="ExternalInput").ap()
    woT = nc.dram_tensor("woT", [128, D], F16, kind="ExternalInput").ap()
    ropeA = nc.dram_tensor("ropeA", [128, NTOK], F16, kind="ExternalInput").ap()
    ropeB = nc.dram_tensor("ropeB", [128, NTOK], F16, kind="ExternalInput").ap()
    outT = nc.dram_tensor("outT", [D, NTOK], F16, kind="ExternalOutput").ap()
    with tile.TileContext(nc) as tc:
        _build_body(tc, xT, wqT, wkT, wvT, woT, ropeA, ropeB, outT)
    nc.compile()
    _NC_CACHE[0] = nc
    return nc


def _rope_tables():
    half = DK // 2  # 32
    inv_freq = 1.0 / (
        10000.0 ** (np.arange(0, DK, 2, dtype=np.float32) / np.float32(DK))
    )
    t = np.arange(T, dtype=np.float32)
    freqs = np.outer(t, inv_freq)  # [T, 32]
    cos = np.cos(freqs)
    sin = np.sin(freqs)
    A = np.empty((128, NTOK), np.float32)
    Bt = np.empty((128, NTOK), np.float32)
    for p in range(128):
        i = p % DK
        if i < half:
            a, bb = cos[:, i], -sin[:, i]
        else:
            a, bb = cos[:, i - half], sin[:, i - half]
        for bi in range(B):
            A[p, bi * T : (bi + 1) * T] = a
            Bt[p, bi * T : (bi + 1) * T] = bb
    return A.astype(np.float16), Bt.astype(np.float16)


def _prep_inputs(x, wq, wk, wv, wo):
    xT = np.ascontiguousarray(x.reshape(NTOK, D).T).astype(np.float16)
    ropeA, ropeB = _rope_tables()
    in_maps = []
    for c in range(N_CORES):
        rows = slice(128 * c, 128 * (c + 1))
        in_maps.append(
            {
                "xT": xT,
                "wqT": np.ascontiguousarray(wq[rows, :].T).astype(np.float16),
                "wkT": np.ascontiguousarray(wk[rows, :].T).astype(np.float16),
                "wvT": np.ascontiguousarray(wv[rows, :].T).astype(np.float16),
                "woT": np.ascontiguousarray(wo[:, rows].T).astype(np.float16),
                "ropeA": ropeA,
                "ropeB": ropeB,
            }
        )
    return in_maps


def run(x, wq, wk, wv, wo, trace=False):
    """Returns (output (B,T,D) fp32, BassKernelResults)."""
    from concourse import bass_utils

    nc = _build_program()
    in_maps = _prep_inputs(
        np.asarray(x, np.float32),
        np.asarray(wq, np.float32),
        np.asarray(wk, np.float32),
        np.asarray(wv, np.float32),
        np.asarray(wo, np.float32),
    )
    res = bass_utils.run_bass_kernel_spmd(
        nc, in_maps, core_ids=list(range(N_CORES)), trace=trace
    )
    acc = np.zeros((D, NTOK), np.float32)
    for c in range(N_CORES):
        acc += np.asarray(res.results[c]["outT"], np.float32)
    out = acc.T.reshape(B, T, D)
    return out, res


def kernel(x, wq, wk, wv, wo):
    out, _ = run(x, wq, wk, wv, wo)
    return out


# revision 24
# speedup vs baseline: 1.1747x; 1.1747x over previous
"""Multi-head attention (RoPE) Trainium2 kernel.

Problem: B=2, T=2048, D_MODEL=1024, 16 heads x d_k=64, fp32 in/out.

Sharding: tensor-parallel over heads. Core c owns heads 2c, 2c+1:
  - wq/wk/wv rows [128c, 128c+128)  (column-split of the projections)
  - wo columns [128c, 128c+128)     (row-split of the output projection)
Each core emits a NORMALIZED fp16 partial of the output projection (softmax
denominators are applied on-chip); the host just sums the 8 partials.

On-chip dataflow per core (fp16 matmul operands, fp32 PSUM):
  xT [D=1024, tok=4096] (token-major b*2048+s) @ wT slices -> QT/KT/VT [128, 4096]
  RoPE on QT/KT in [d', tok] layout (tables precomputed host-side; the +-32
  partition swap is a SBUF-SBUF DMA).
  V transposed per 128-token tile on the PE to [tok, 64]-per-head tiles with
  a ones column appended (the 65th stationary column accumulates the softmax
  denominator into PSUM row 64 of the AV accumulator for free).
  Attention runs in 512-query chunks (8 chunks of (b, qi)).  Per key tile kt:
  both heads' score matmuls are row-tiled ((0,0)/(64,0)) into ONE [128,1024]
  PSUM tile (head h at columns h*512:(h+1)*512), so a single FD=1024 exp on
  ScalarE covers both heads.  Score tiles rotate through 3 PSUM slots so the
  PE can run ~2 key-tiles ahead of the exp stream.
  After the AV accumulation, row 64 of each accumulator holds the softmax
  denominator; a DVE reciprocal + a tiny K=2 PE matmul broadcasts 1/den to
  all 128 partitions, one tensor_mul normalizes the concatenated [128,512]
  attention output, and a single K=128 output projection (both heads fused)
  produces this core's fp16 partial, evicted with one fused DMA per chunk.

ScalarE (exp: 128 tiles x ~1.15us = ~147us) is the critical engine; all PE
work that is not on the scores->exp->AV chain (projections, RoPE, V
transposes) is streamed through a fill queue into the gaps of the attention
kt loop, and the previous chunk's output projection runs as a burst at each
chunk start, so the PE never idles long enough for the HAM clock gate to
re-throttle it to 1.2 GHz.
"""

import sys

sys.path.insert(0, "/opt/trn_rl_repo")

from collections import deque

import numpy as np

import concourse.bacc as bacc
import concourse.bass as bass
import concourse.tile as tile
from concourse import mybir
from concourse.masks import make_identity

F16 = mybir.dt.float16
F32 = mybir.dt.float32

B = 2
T = 2048
D = 1024
NTOK = B * T  # 4096
DK = 64
N_CORES = 8
QCH = 512  # query chunk
NCHUNK = NTOK // QCH  # 8
KT_N = T // 128  # 16 key tiles per batch


def _build_body(tc, xT, wqT, wkT, wvT, woT, ropeA, ropeB, outT):
    nc = tc.nc
    Exp = mybir.ActivationFunctionType.Exp

    const = tc.alloc_tile_pool(name="const", bufs=1)
    psum = tc.alloc_tile_pool(name="psum", bufs=1, space="PSUM")
    at = tc.alloc_tile_pool(name="attn", bufs=1)
    pp = tc.alloc_tile_pool(name="phasep", bufs=1)

    # ---------------- persistent tiles ----------------
    # Warm the ScalarE Exp table set (~2.7us) during the preamble DMAs.
    warm = const.tile([1, 8], F32)
    nc.vector.memset(warm, 0.0)
    warm_o = const.tile([1, 8], F16)
    nc.scalar.activation(warm_o, warm, Exp, scale=0.125)

    w_sb = {}
    for nm, w in (("wk", wkT), ("wq", wqT), ("wv", wvT)):
        wt = const.tile([128, 8, 128], F16, name=f"{nm}sb")
        nc.sync.dma_start(out=wt, in_=w.rearrange("(a p) m -> p a m", p=128))
        w_sb[nm] = wt
    ident = const.tile([128, 128], F16)
    make_identity(nc, ident)
    # den-broadcast selector: out[m,q] = sel[0,m]*rcp0[q] + sel[1,m]*rcp1[q]
    # (row 1 lives at partition 1, which DVE can't address -> build via DMA)
    sel = const.tile([2, 128], F32)
    ones128 = const.tile([1, 128], F32)
    nc.vector.memset(ones128, 1.0)
    nc.vector.memset(sel, 0.0)
    nc.sync.dma_start(out=sel[0:1, 0:64], in_=ones128[0:1, 0:64])
    nc.sync.dma_start(out=sel[1:2, 64:128], in_=ones128[0:1, 64:128])

    q_rot = const.tile([128, 4096], F16)
    k_rot = const.tile([128, 4096], F16)
    # per 128-token tile, per head: [V(0:64) | ones(64) | pad] fp16, all in
    # one big tile (slot 2*g+h) so a single memset plants the ones columns.
    vbig = const.tile([128, 64, 72], F16, name="vbig")
    nc.vector.memset(vbig, 1.0)

    def v_sb(g, h):
        return vbig[:, 2 * g + h, :]

    # x in SBUF: 8 k-tiles of [128, 4096], loaded in phase-sized quarters
    # (phase-0 projections start after the first 8 x 256 KB arrive).  The
    # RoPE tables repeat with period 64 partitions, so only half is read
    # from HBM; a SBUF-SBUF DMA duplicates it.  wo is not needed until the
    # first output projection (chunk 2) and loads last.
    xs = [pp.tile([128, 4096], F16, name=f"xs{k}") for k in range(8)]
    rA = const.tile([128, 4096], F16)
    rB = const.tile([128, 4096], F16)
    for k in range(8):
        nc.sync.dma_start(out=xs[k][:, 0:2048], in_=xT[k * 128 : (k + 1) * 128, 0:2048])
    nc.sync.dma_start(out=rA, in_=ropeA)
    nc.sync.dma_start(out=rB, in_=ropeB)
    for k in range(8):
        nc.sync.dma_start(
            out=xs[k][:, 2048:4096], in_=xT[k * 128 : (k + 1) * 128, 2048:4096]
        )
    wo_sb = const.tile([128, 1024], F16)
    nc.sync.dma_start(out=wo_sb, in_=woT)

    vt_raw = [pp.tile([128, 1024], F16, name=f"vtr{i}") for i in range(4)]

    # ---------------- projection / RoPE / V-transpose units ----------------
    def proj_quarter(wt, dst, t4, j, dst_off=None):
        # 4 matmuls: k in {2j, 2j+1} x h2; the accumulation group spans the
        # four quarters (start at k==0, stop at k==7); evict on the last.
        cs0 = t4 * 1024
        do = cs0 if dst_off is None else dst_off
        if j == 0:
            proj_quarter.cur = psum.tile(
                [128, 1024], F32, tag="proj", bufs=1, name="ps_pr"
            )
        ps = proj_quarter.cur
        for k in (2 * j, 2 * j + 1):
            for h2 in range(2):
                nc.tensor.matmul(
                    ps[:, h2 * 512 : (h2 + 1) * 512],
                    lhsT=wt[:, k, :],
                    rhs=xs[k][:, cs0 + h2 * 512 : cs0 + (h2 + 1) * 512],
                    start=(k == 0),
                    stop=(k == 7),
                    skip_group_check=True,
                )
        if j == 3:
            nc.vector.tensor_copy(dst[:, do : do + 1024], ps)

    def rope_chunk(raw, t4):
        # out = raw*A + swap(raw)*B, swap = +-32 partitions within a head
        cs = slice(t4 * 1024, (t4 + 1) * 1024)
        sw = pp.tile([128, 1024], F16, tag="sw", bufs=2, name="ropesw")
        for dst_p, src_p in ((0, 32), (32, 0), (64, 96), (96, 64)):
            nc.sync.dma_start(
                out=sw[dst_p : dst_p + 32, :], in_=raw[src_p : src_p + 32, cs]
            )
        t1 = pp.tile([128, 1024], F16, tag="t1", bufs=2, name="ropet1")
        nc.vector.tensor_mul(t1, raw[:, cs], rA[:, cs])
        nc.vector.tensor_mul(sw, sw, rB[:, cs])
        nc.vector.tensor_add(raw[:, cs], t1, sw)

    def v_transpose(t4, i):
        # vt_raw[t4] [d', tok] -> v_sb [tok128, d64] for token tile 8*t4+i
        pst = psum.tile([128, 1024], F32, tag="mm", bufs=2, name="ps_tr")
        tr = pst[:, 0:64].bitcast(F16)  # [128, 128] f16 view
        nc.tensor.transpose(tr, vt_raw[t4][:, i * 128 : (i + 1) * 128], ident)
        g = 8 * t4 + i
        nc.vector.tensor_copy(v_sb(g, 0)[:, 0:64], tr[:, 0:64])
        nc.vector.tensor_copy(v_sb(g, 1)[:, 0:64], tr[:, 64:128])

    # ---------------- fill + oproj queues ----------------
    # Every fill unit carries a (t4, part) key; `require` force-drains the
    # queue until the named parts are fully EMITTED, so a chunk's score/AV
    # instructions are never emitted before the projections they read --
    # emission order is what Tile derives dependencies from.
    fillq = deque()
    oq = deque()
    remaining = {}

    def fill(n):
        for _ in range(n):
            if not fillq:
                return
            key, fn = fillq.popleft()
            fn()
            if key is not None:
                remaining[key] -= 1

    def require(*keys):
        while any(remaining.get(k, 0) > 0 for k in keys):
            fill(1)

    def put(key, fn):
        remaining[key] = remaining.get(key, 0) + 1
        fillq.append((key, fn))

    def enqueue_phase(t4, parts):
        for p in parts:
            key = (t4, p)
            if p == "k":
                for j in range(4):
                    put(key, lambda j=j: proj_quarter(w_sb["wk"], k_rot, t4, j))
                put(key, lambda: rope_chunk(k_rot, t4))
            elif p == "q":
                for j in range(4):
                    put(key, lambda j=j: proj_quarter(w_sb["wq"], q_rot, t4, j))
                put(key, lambda: rope_chunk(q_rot, t4))
            elif p == "v":
                for j in range(4):
                    put(
                        key,
                        lambda j=j: proj_quarter(w_sb["wv"], vt_raw[t4], t4, j, dst_off=0),
                    )
                for i in range(0, 8, 2):
                    put(key, lambda i=i: (v_transpose(t4, i), v_transpose(t4, i + 1)))

    # ---------------- attention ----------------
    def chunk(c):
        b, qi = c // 4, c % 4
        qoff = b * T + qi * QCH

        exp_tiles = {}

        def s_exp(kt):
            koff = b * T + kt * 128
            pss = psum.tile([128, 1024], F32, tag="mm", bufs=2, name="ps_s")
            for hi in range(2):
                hs = slice(64 * hi, 64 * hi + 64)
                nc.tensor.matmul(
                    pss[:, hi * 512 : (hi + 1) * 512],
                    lhsT=k_rot[hs, koff : koff + 128],
                    rhs=q_rot[hs, qoff : qoff + QCH],
                    start=True,
                    stop=True,
                )
            e = at.tile([128, 1024], F16, tag="exp", bufs=4, name="exps")
            nc.scalar.activation(e, pss, Exp, scale=0.125)
            exp_tiles[kt] = e

        ps_o = []

        def av(kt):
            g = b * KT_N + kt
            e = exp_tiles.pop(kt)
            for hi in range(2):
                nc.tensor.matmul(
                    ps_o[hi],
                    lhsT=v_sb(g, hi)[:, 0:65],
                    rhs=e[:, hi * 512 : (hi + 1) * 512],
                    start=(kt == 0),
                    stop=(kt == KT_N - 1),
                    skip_group_check=True,
                )

        require((2 * b, "k"), (c // 2, "q"))
        s_exp(0)
        s_exp(1)
        # prior chunks' output-projection units run as fill work two chunks
        # later, when the den/normalize chain is guaranteed long-finished, so
        # no PE instruction in this chunk's queue ever head-of-line blocks.
        ndrain = len(oq) if c == NCHUNK - 1 else (1 if len(oq) >= 2 else 0)
        for _ in range(ndrain):
            for u in reversed(oq.popleft()):
                fillq.appendleft((None, u))
        ps_o.extend(
            psum.tile([65, QCH], F32, tag="o", bufs=2, name=f"ps_o{hi}")
            for hi in range(2)
        )
        # AV trails scores by TWO key-tiles so its exp input is always long
        # ready -- the PE never waits on ScalarE, keeping matmuls
        # back-to-back (streamed ~N/2.4 instead of isolated (398+N)/2.4).
        require((2 * b, "v"))
        s_exp(2)
        av(0)
        fill(1)
        for kt in range(3, KT_N):
            if kt == 8:
                require((2 * b + 1, "k"))
            if kt == 10:
                require((2 * b + 1, "v"))
            s_exp(kt)
            av(kt - 2)
            fill(2 if c >= NCHUNK - 2 else 1)
        av(KT_N - 2)
        av(KT_N - 1)

        # softmax denominators -> 1/den broadcast -> normalize -> oproj
        # denominator rows -> SBUF, spread across 128 partitions, one wide
        # reciprocal (8 elem/lane instead of 512 on one lane), then back into
        # the [2, 512] layout the broadcast matmul wants.
        dT = at.tile([128, 8], F32, tag="dT", bufs=3, name="dT")
        for hi in range(2):
            dtmp = at.tile([1, QCH], F32, tag=f"dtmp{hi}", bufs=3, name="dtmp")
            nc.vector.tensor_copy(dtmp, ps_o[hi][64:65, :])
            nc.sync.dma_start(out=dT[:, 4 * hi : 4 * hi + 4], in_=dtmp)
        rT = at.tile([128, 8], F32, tag="rT", bufs=3, name="rT")
        nc.vector.reciprocal(rT, dT)
        rcp2 = at.tile([2, QCH], F32, tag="rcp", bufs=3, name="rcp2")
        for hi in range(2):
            nc.sync.dma_start(out=rcp2[hi : hi + 1, :], in_=rT[:, 4 * hi : 4 * hi + 4])
        ocat = at.tile([128, QCH], F16, tag="ocat", bufs=3, name="ocat")
        nc.vector.tensor_copy(ocat[0:64, :], ps_o[0][0:64, :])
        oBt = at.tile([64, QCH], F16, tag="oBt", bufs=3, name="oBt")
        nc.vector.tensor_copy(oBt, ps_o[1][0:64, :])
        nc.sync.dma_start(out=ocat[64:128, :], in_=oBt)

        st = {}

        def o_pre():
            ps_b = psum.tile([128, QCH], F32, tag="mm", bufs=2, name="ps_b")
            nc.tensor.matmul(ps_b, lhsT=sel, rhs=rcp2, start=True, stop=True)
            ocn = at.tile([128, QCH], F16, tag="ocn", bufs=3, name="ocn")
            nc.vector.tensor_mul(ocn, ocat, ps_b)
            st["ocn"] = ocn
            st["ot"] = at.tile([128, 8, QCH], F16, tag="ot", bufs=3, name="ot")

        def o_nt(nt):
            ps_u = psum.tile([128, QCH], F32, tag="mm", bufs=2, name="ps_u")
            nc.tensor.matmul(
                ps_u,
                lhsT=wo_sb[:, nt * 128 : (nt + 1) * 128],
                rhs=st["ocn"],
                start=True,
                stop=True,
            )
            nc.vector.tensor_copy(st["ot"][:, nt, :], ps_u)
            if nt == 7:
                nc.sync.dma_start(
                    out=outT[:, qoff : qoff + QCH].rearrange(
                        "(nt p) q -> p nt q", nt=8
                    ),
                    in_=st["ot"],
                )

        oq.append([o_pre] + [(lambda nt=nt: o_nt(nt)) for nt in range(8)])

    # ---------------- schedule ----------------
    # Preamble: phase 0 fully + phase 1 K; phase 1 V leads the fill queue so
    # batch 0's second-half V tiles are ready by chunk 0's kt 8.
    for j in range(4):
        proj_quarter(w_sb["wk"], k_rot, 0, j)
    rope_chunk(k_rot, 0)
    for j in range(4):
        proj_quarter(w_sb["wq"], q_rot, 0, j)
    rope_chunk(q_rot, 0)
    for j in range(4):
        proj_quarter(w_sb["wv"], vt_raw[0], 0, j, dst_off=0)
    for i in range(0, 8, 2):
        v_transpose(0, i)
        v_transpose(0, i + 1)
    enqueue_phase(1, "kv")
    enqueue_phase(1, "q")
    enqueue_phase(2, "kv")
    enqueue_phase(2, "q")
    enqueue_phase(3, "kv")
    enqueue_phase(3, "q")

    for c in range(NCHUNK):
        chunk(c)
        if c == NCHUNK - 1:
            fill(10**9)
            while oq:
                for u in oq.popleft():
                    u()

    pp.release()
    at.release()
    const.release()
    psum.release()


_NC_CACHE = {}


def _build_program():
    if 0 in _NC_CACHE:
        return _NC_CACHE[0]
    nc = bacc.Bacc("TRN2", num_devices=N_CORES, debug=False)
    xT = nc.dram_tensor("xT", [D, NTOK], F16, kind="ExternalInput").ap()
    wqT = nc.dram_tensor("wqT", [D, 128], F16, kind="ExternalInput").ap()
    wkT = nc.dram_tensor("wkT", [D, 128], F16, kind="ExternalInput").ap()
    wvT = nc.dram_tensor("wvT", [D, 128], F16, kind="ExternalInput").ap()
    woT = nc.dram_tensor("woT", [128, D], F16, kind="ExternalInput").ap()
    ropeA = nc.dram_tensor("ropeA", [128, NTOK], F16, kind="ExternalInput").ap()
    ropeB = nc.dram_tensor("ropeB", [128, NTOK], F16, kind="ExternalInput").ap()
    outT = nc.dram_tensor("outT", [D, NTOK], F16, kind="ExternalOutput").ap()
    with tile.TileContext(nc) as tc:
        _build_body(tc, xT, wqT, wkT, wvT, woT, ropeA, ropeB, outT)
    nc.compile()
    _NC_CACHE[0] = nc
    return nc


def _rope_tables():
    half = DK // 2  # 32
    inv_freq = 1.0 / (
        10000.0 ** (np.arange(0, DK, 2, dtype=np.float32) / np.float32(DK))
    )
    t = np.arange(T, dtype=np.float32)
    freqs = np.outer(t, inv_freq)  # [T, 32]
    cos = np.cos(freqs)
    sin = np.sin(freqs)
    A = np.empty((128, NTOK), np.float32)
    Bt = np.empty((128, NTOK), np.float32)
    for p in range(128):
        i = p % DK
        if i < half:
            a, bb = cos[:, i], -sin[:, i]
        else:
            a, bb = cos[:, i - half], sin[:, i - half]
        for bi in range(B):
            A[p, bi * T : (bi + 1) * T] = a
            Bt[p, bi * T : (bi + 1) * T] = bb
    return A.astype(np.float16), Bt.astype(np.float16)


def _prep_inputs(x, wq, wk, wv, wo):
    xT = np.ascontiguousarray(x.reshape(NTOK, D).T).astype(np.float16)
    ropeA, ropeB = _rope_tables()
    in_maps = []
    for c in range(N_CORES):
        rows = slice(128 * c, 128 * (c + 1))
        in_maps.append(
            {
                "xT": xT,
                "wqT": np.ascontiguousarray(wq[rows, :].T).astype(np.float16),
                "wkT": np.ascontiguousarray(wk[rows, :].T).astype(np.float16),
                "wvT": np.ascontiguousarray(wv[rows, :].T).astype(np.float16),
                "woT": np.ascontiguousarray(wo[:, rows].T).astype(np.float16),
                "ropeA": ropeA,
                "ropeB": ropeB,
            }
        )
    return in_maps


def run(x, wq, wk, wv, wo, trace=False):
    """Returns (output (B,T,D) fp32, BassKernelResults)."""
    from concourse import bass_utils

    nc = _build_program()
    in_maps = _prep_inputs(
        np.asarray(x, np.float32),
        np.asarray(wq, np.float32),
        np.asarray(wk, np.float32),
        np.asarray(wv, np.float32),
        np.asarray(wo, np.float32),
    )
    res = bass_utils.run_bass_kernel_spmd(
        nc, in_maps, core_ids=list(range(N_CORES)), trace=trace
    )
    acc = np.zeros((D, NTOK), np.float32)
    for c in range(N_CORES):
        acc += np.asarray(res.results[c]["outT"], np.float32)
    out = acc.T.reshape(B, T, D)
    return out, res


def kernel(x, wq, wk, wv, wo):
    out, _ = run(x, wq, wk, wv, wo)
    return out
